# revision 8
# baseline (speedup 1.0000x reference)
"""MoE (top-2 of 8 experts, SwiGLU) kernel for 8 TRN2 NeuronCores.

Expert-parallel with a single AllToAll combine. Core e holds expert e's
weights resident in SBUF and computes rows only for tokens routed to e
(host-side gather). Each token's two expert contributions are summed on the
COMBINER core chosen per token by a balancing router that minimizes padded
row counts. Per-core slot layout (identical, compile-time, on all cores):

  [ send 8*S | keep kp_off[0]..kp_off[7] | pad ]      C_pad rows total

Send segment d (S rows) holds wcg-scaled rows this core computed for tokens
whose combiner is core d; segment e holds this core's self-pair tokens (the
A2A returns them in place). One AllToAll (fired as soon as the send region
is fully written, hiding under the keep-region compute) moves segment d to
core d. Keep group s holds the rows this core combines with recv segment s:
group s row j <-> recv row s*S+j (the self group has wcg=0 slots, so the
recv diag supplies the whole contribution). Keep rows are written wcg-scaled
to a DRAM stash; after the A2A a short combine pass reads stash + recv and
adds them on the Pool engine (DVE stays free for the silu*up fusion), so the
only post-collective work is a few tile adds + output DMAs.

All slot addressing is compile-time and identical across cores; per-core
variation lives in the data (hsTg column gather + wcg weights). Pad slots
carry zero hs columns and wcg=0 so they compute to exact zeros.

Gate/up matmuls stream 512-wide (PSUM-bank max; measured optimal at
~0.53ns/row). The <512 tail block interleaves the gate/up chains to dodge
the ~300ns narrow-matmul latency floor. Weight loads are split into
h-quarters and interleaved with the first block's chains. Matmul operands
are bf16 (fp32 PSUM accumulation).
"""

import numpy as np
import ml_dtypes

import jax
import concourse.bass as bass
import concourse.tile as tile
from concourse import bacc, mybir
from concourse.bass import ts

E, H, I, T, KTOP = 8, 2048, 1408, 4096, 2
NCORES = 8

BF16 = mybir.dt.bfloat16
F32 = mybir.dt.float32


def _ceil128(x):
    return (x + 127) // 128 * 128


def _layout_sizes(M, self_cnt):
    """M[e][d] = rows e computes for combiner d (d!=e). Returns S_seg, KP.
    Self tokens live in keep group e with full weight; the A2A diag segment
    is all zero-pads, so S_seg is bound only by cross-pair counts."""
    s_off = max(M[e][d] for e in range(E) for d in range(E) if e != d)
    KP = []
    for s in range(E):
        KP.append(max(self_cnt[s],
                      max(M[s][d] for d in range(E) if d != s)))
    return s_off, KP


def _balance(pair_n, self_cnt):
    """Choose cnt[x][y] (tokens of pair {x,y} combined by y) to minimize
    C_pad = pad128(8*S + sum(KP)); S/KP as in _layout_sizes."""
    M = [[0] * E for _ in range(E)]
    pairs = list(pair_n.keys())
    for (x, y) in pairs:
        n = pair_n[(x, y)]
        M[x][y] = n // 2
        M[y][x] = n - n // 2

    def cost():
        S, KP = _layout_sizes(M, self_cnt)
        c = 8 * S + sum(KP)
        return (_ceil128(c), c)

    best = cost()
    improved = True
    while improved:
        improved = False
        for (x, y) in pairs:
            for (a, b) in ((x, y), (y, x)):
                for delta in (16, 8, 4, 2, 1):
                    if M[a][b] >= delta:
                        M[a][b] -= delta
                        M[b][a] += delta
                        c = cost()
                        if c < best:
                            best = c
                            improved = True
                        else:
                            M[a][b] += delta
                            M[b][a] -= delta
    return M


def _route(hidden_states, top_k_index, top_k_weights):
    """Host-side routing. Returns per-core in_maps (hsTg, wcg), the layout
    (S_seg, KP tuple), and per-core keep lists for host assembly."""
    hs = np.asarray(hidden_states, dtype=np.float32)
    idx = np.asarray(top_k_index).astype(np.int64)
    tw = np.asarray(top_k_weights, dtype=np.float32)

    w = np.zeros((E, T), dtype=np.float32)
    tarange = np.arange(T)
    for k in range(KTOP):
        np.add.at(w, (idx[:, k], tarange), tw[:, k])

    a = np.minimum(idx[:, 0], idx[:, 1])
    b = np.maximum(idx[:, 0], idx[:, 1])
    pair_tokens = {}
    self_toks = [[] for _ in range(E)]
    for t in range(T):
        x, y = int(a[t]), int(b[t])
        if x == y:
            self_toks[x].append(t)
        else:
            pair_tokens.setdefault((x, y), []).append(t)

    pair_n = {p: len(v) for p, v in pair_tokens.items()}
    self_cnt = [len(s) for s in self_toks]
    M = _balance(pair_n, self_cnt)

    # sendlist[e][d]: tokens e computes whose combiner is d (d==e: self,
    # keep-only — the A2A diag segment carries zeros)
    sendlist = [[[] for _ in range(E)] for _ in range(E)]
    for (x, y), toks in pair_tokens.items():
        nxy = M[x][y]
        sendlist[x][y] = toks[:nxy]
        sendlist[y][x] = toks[nxy:]
    for e in range(E):
        sendlist[e][e] = self_toks[e]

    S_seg, KP = _layout_sizes(M, self_cnt)
    SND = NCORES * S_seg
    kp_off = np.concatenate([[0], np.cumsum(KP)]).astype(int)
    C = SND + int(kp_off[-1])
    C_pad = _ceil128(C)

    hsT_bf = np.ascontiguousarray(hs.T).astype(ml_dtypes.bfloat16)
    in_maps, plans = [], []
    for e in range(E):
        cols = np.full(C_pad, -1, dtype=np.int64)
        wcg = np.zeros(C_pad, dtype=np.float32)
        for d in range(E):
            if d == e:
                continue  # diag segment stays all zero-pads
            L = sendlist[e][d]
            cols[d * S_seg:d * S_seg + len(L)] = L
            wcg[d * S_seg:d * S_seg + len(L)] = w[e, L]
        keep_lists = []
        for s in range(E):
            L = sendlist[s][e]
            pos = SND + kp_off[s]
            cols[pos:pos + len(L)] = L
            # self group (s == e) carries its full weight; its recv-diag
            # add contributes exact zeros
            wcg[pos:pos + len(L)] = w[e, L]
            keep_lists.append(L)
        mask = cols < 0
        cidx = np.where(mask, 0, cols)
        g = hsT_bf[:, cidx]
        g[:, mask] = 0
        in_maps.append({"hsTg": np.ascontiguousarray(g), "wcg": wcg})
        plans.append(keep_lists)
    return in_maps, (S_seg, tuple(KP)), plans


def _build_moe(S_seg, KP, h=H, i_sz=I, ncores=NCORES, use_a2a=True):
    SND = ncores * S_seg
    kp_off = [0]
    for k in KP:
        kp_off.append(kp_off[-1] + k)
    KEEP = kp_off[-1]
    C = SND + KEEP
    C_pad = _ceil128(C)
    OUT_ROWS = C_pad - SND
    ntiles = C_pad // 128
    fire_tile = (SND - 1) // 128

    hc, ic2 = h // 128, i_sz // 128
    hh = hc // 4  # h-chunk quarter for interleaved weight loads

    blocks = []
    pos = 0
    while C_pad - pos > 512:
        blocks.append((pos, 512))
        pos += 512
    if C_pad - pos:
        blocks.append((pos, C_pad - pos))

    nc = bacc.Bacc("TRN2", target_bir_lowering=False, debug=False,
                   num_devices=ncores)
    hsTg = nc.declare_dram_parameter("hsTg", [h, C_pad], BF16, isOutput=False).ap()
    wg = nc.declare_dram_parameter("wg", [h, i_sz], BF16, isOutput=False).ap()
    wu = nc.declare_dram_parameter("wu", [h, i_sz], BF16, isOutput=False).ap()
    wd = nc.declare_dram_parameter("wd", [i_sz, h], BF16, isOutput=False).ap()
    wcg = nc.declare_dram_parameter("wcg", [C_pad], F32, isOutput=False).ap()
    out = nc.declare_dram_parameter("out", [OUT_ROWS, h], BF16, isOutput=True).ap()

    silu = mybir.ActivationFunctionType.Silu
    world = [list(range(ncores))]

    # keep-group spans per combine tile, in OUT-row coordinates. Clamp to
    # the recv segment size: group rows past S_seg can only be self-pad
    # rows, which take no recv contribution.
    def rcv_spans(r0, rows):
        spans = []
        for s in range(ncores):
            a = max(r0, kp_off[s])
            b = min(r0 + rows, kp_off[s + 1], kp_off[s] + S_seg)
            if a < b:
                spans.append((a - r0, b - r0, s * S_seg + a - kp_off[s]))
        return spans

    with tile.TileContext(nc) as tc:
        with (
            tc.tile_pool(name="wpool", bufs=1) as wpool,
            tc.tile_pool(name="hspool", bufs=2) as hspool,
            tc.tile_pool(name="apool", bufs=1) as apool,
            tc.tile_pool(name="stage", bufs=2) as stage,
            tc.tile_pool(name="ypool", bufs=3) as ypool,
            tc.tile_pool(name="rpool", bufs=3) as rpool,
            tc.tile_pool(name="pg", bufs=2, space="PSUM") as pg,
            tc.tile_pool(name="pu", bufs=2, space="PSUM") as pu,
            tc.tile_pool(name="py", bufs=4, space="PSUM") as py,
            tc.tile_pool(name="dram", bufs=1, space="DRAM") as dram,
        ):
            # block 0's hidden states first in the DMA queue, then weight
            # halves in the order the first matmul chains consume them.
            (pos0, nb0) = blocks[0]
            hs0 = hspool.tile([128, hc, nb0], BF16, tag="hst")
            nc.sync.dma_start(
                out=hs0[:],
                in_=hsTg[:, pos0:pos0 + nb0].rearrange("(c p) t -> p c t", p=128))

            wg_h = [wpool.tile([128, hh, i_sz], BF16, name=f"wg{i}",
                               tag=f"wg{i}") for i in range(4)]
            wu_h = [wpool.tile([128, hh, i_sz], BF16, name=f"wu{i}",
                               tag=f"wu{i}") for i in range(4)]
            for i in range(4):
                nc.sync.dma_start(
                    out=wg_h[i][:],
                    in_=wg[i * hh * 128:(i + 1) * hh * 128, :]
                    .rearrange("(c p) i -> p c i", p=128))
                nc.sync.dma_start(
                    out=wu_h[i][:],
                    in_=wu[i * hh * 128:(i + 1) * hh * 128, :]
                    .rearrange("(c p) i -> p c i", p=128))
            wd_sb = wpool.tile([128, ic2, h], BF16, tag="wd")
            nc.sync.dma_start(out=wd_sb[:], in_=wd.rearrange("(c p) j -> p c j", p=128))
            wcg_sb = wpool.tile([128, ntiles], F32, tag="wcg")
            nc.sync.dma_start(out=wcg_sb[:], in_=wcg.rearrange("(ct p) -> p ct", p=128))
            # zero tile: DMA-source for partition ranges engine ops can't
            # slice (walrus rejects partition-sliced elementwise ops)
            zsb = wpool.tile([128, h], BF16, tag="zsb")
            nc.vector.memset(zsb[:], 0.0)

            sendbuf = dram.tile([SND, h], BF16, tag="sendbuf")
            recvbuf = dram.tile([SND, h], BF16, tag="recvbuf")
            stash = dram.tile([OUT_ROWS, h], BF16, tag="stash")

            for bi, (pos, nb) in enumerate(blocks):
                if bi == 0:
                    hs_t = hs0
                else:
                    hs_t = hspool.tile([128, hc, nb], BF16, tag="hst")
                    nc.sync.dma_start(
                        out=hs_t[:],
                        in_=hsTg[:, pos:pos + nb].rearrange("(c p) t -> p c t", p=128))

                aT = apool.tile([128, ic2, nb], BF16, tag="aT")
                interleave = nb < 512
                for it in range(ic2):
                    psg = pg.tile([128, nb], F32, tag="psg")
                    psu = pu.tile([128, nb], F32, tag="psu")
                    if interleave:
                        for c in range(hc):
                            half, cc = c // hh, c % hh
                            nc.tensor.matmul(
                                psg[:], lhsT=wg_h[half][:, cc, ts(it, 128)],
                                rhs=hs_t[:, c, :],
                                start=(c == 0), stop=(c == hc - 1))
                            nc.tensor.matmul(
                                psu[:], lhsT=wu_h[half][:, cc, ts(it, 128)],
                                rhs=hs_t[:, c, :],
                                start=(c == 0), stop=(c == hc - 1))
                    else:
                        for half in range(4):
                            for cc in range(hh):
                                c = half * hh + cc
                                nc.tensor.matmul(
                                    psg[:], lhsT=wg_h[half][:, cc, ts(it, 128)],
                                    rhs=hs_t[:, c, :],
                                    start=(c == 0), stop=(c == hc - 1))
                            for cc in range(hh):
                                c = half * hh + cc
                                nc.tensor.matmul(
                                    psu[:], lhsT=wu_h[half][:, cc, ts(it, 128)],
                                    rhs=hs_t[:, c, :],
                                    start=(c == 0), stop=(c == hc - 1))
                    sil = stage.tile([128, nb], F32, tag="sil")
                    nc.scalar.activation(out=sil[:], in_=psg[:], func=silu)
                    nc.vector.tensor_mul(aT[:, it, :], sil[:], psu[:])

                for ct in range(nb // 128):
                    gct = pos // 128 + ct
                    g0 = gct * 128

                    y_sb = ypool.tile([128, h], BF16, tag="ysb")
                    for hb in range(h // 512):
                        psy = py.tile([128, 512], F32, tag="psy")
                        for c2 in range(ic2):
                            nc.tensor.matmul(psy[:],
                                             lhsT=aT[:, c2, ts(ct, 128)],
                                             rhs=wd_sb[:, c2, ts(hb, 512)],
                                             start=(c2 == 0),
                                             stop=(c2 == ic2 - 1))
                        nc.vector.tensor_scalar_mul(
                            y_sb[:, ts(hb, 512)], psy[:],
                            wcg_sb[:, gct:gct + 1])

                    snd_rows = min(max(SND - g0, 0), 128)
                    if snd_rows > 0:
                        nc.sync.dma_start(out=sendbuf[g0:g0 + snd_rows, :],
                                          in_=y_sb[0:snd_rows, :])
                    if snd_rows < 128:
                        r0 = g0 + snd_rows - SND
                        nc.sync.dma_start(
                            out=stash[r0:r0 + 128 - snd_rows, :],
                            in_=y_sb[snd_rows:128, :])

                    if use_a2a and gct == fire_tile:
                        nc.gpsimd.collective_compute(
                            "AllToAll", mybir.AluOpType.bypass,
                            replica_groups=world,
                            ins=[sendbuf[:].opt()], outs=[recvbuf[:].opt()])

            # combine pass: out = stash + recv (Pool engine). Engine ops are
            # full-tile only; uncovered partition ranges are DMA-zero-filled.
            r0 = 0
            while r0 < OUT_ROWS:
                rows = min(128, OUT_ROWS - r0)
                spans = rcv_spans(r0, rows)
                y2 = ypool.tile([128, h], BF16, tag="ysb")
                nc.sync.dma_start(out=y2[0:rows, :], in_=stash[r0:r0 + rows, :])
                if rows < 128:
                    nc.sync.dma_start(out=y2[rows:128, :], in_=zsb[rows:128, :])
                y3 = ypool.tile([128, h], BF16, tag="ysb")
                if spans:
                    rcv = rpool.tile([128, h], BF16, tag="rcv")
                    for (p0, p1, rr) in spans:
                        nc.sync.dma_start(out=rcv[p0:p1, :],
                                          in_=recvbuf[rr:rr + (p1 - p0), :])
                    # zero-fill partition gaps between/around spans
                    covered = sorted((p0, p1) for (p0, p1, _) in spans)
                    cur = 0
                    gaps = []
                    for (p0, p1) in covered:
                        if p0 > cur:
                            gaps.append((cur, p0))
                        cur = max(cur, p1)
                    if cur < 128:
                        gaps.append((cur, 128))
                    for (p0, p1) in gaps:
                        nc.sync.dma_start(out=rcv[p0:p1, :],
                                          in_=zsb[p0:p1, :])
                    nc.gpsimd.tensor_add(y3[:], y2[:], rcv[:])
                else:
                    nc.gpsimd.tensor_copy(y3[:], y2[:])
                nc.sync.dma_start(out=out[r0:r0 + rows, :], in_=y3[0:rows, :])
                r0 += rows

    nc.compile()
    return nc


class _Runner:
    """Compile once, execute many. Mirrors bass2jax.run_bass_via_pjrt's
    multi-core path but keeps the jitted callable (and device-resident
    inputs) alive so repeat executions skip XLA/NEFF compilation."""

    def __init__(self, nc, n_cores):
        from concourse import bass2jax, mybir as _mybir
        from jax.experimental.shard_map import shard_map
        from jax.sharding import Mesh, PartitionSpec

        bass2jax.install_neuronx_cc_hook()
        partition_name = (nc.partition_id_tensor.name
                          if nc.partition_id_tensor else None)

        in_names, out_names, out_avals, zero_outs = [], [], [], []
        for alloc in nc.m.functions[0].allocations:
            if not isinstance(alloc, _mybir.MemoryLocationSet):
                continue
            name = alloc.memorylocations[0].name
            if alloc.kind == "ExternalInput":
                if name != partition_name:
                    in_names.append(name)
            elif alloc.kind == "ExternalOutput":
                shape = tuple(alloc.tensor_shape)
                dtype = _mybir.dt.np(alloc.dtype)
                out_names.append(name)
                out_avals.append(jax.core.ShapedArray(shape, dtype))
                zero_outs.append(np.zeros(shape, dtype))
        self.n_params = len(in_names)
        self.param_names = list(in_names)
        self.out_names = out_names
        self.out_avals = out_avals
        self.n_cores = n_cores
        all_names = in_names + out_names
        if partition_name is not None:
            all_names.append(partition_name)

        def _body(*args):
            operands = list(args)
            if partition_name is not None:
                operands.append(bass2jax.partition_id_tensor())
            outs = bass2jax._bass_exec_p.bind(
                *operands,
                out_avals=tuple(out_avals),
                in_names=tuple(all_names),
                out_names=tuple(out_names),
                lowering_input_output_aliases=(),
                sim_require_finite=True,
                sim_require_nnan=True,
                nc=nc,
            )
            return tuple(outs)

        devices = jax.devices()[:n_cores]
        assert len(devices) == n_cores
        mesh = Mesh(np.asarray(devices), ("core",))
        n_ops = self.n_params + len(out_names)
        self._body = _body
        self._mesh = mesh
        self._in_specs = (PartitionSpec("core"),) * n_ops
        self._out_specs = (PartitionSpec("core"),) * len(out_names)
        self._fn = jax.jit(
            shard_map(_body, mesh=mesh,
                      in_specs=self._in_specs,
                      out_specs=self._out_specs,
                      check_rep=False),
            keep_unused=True)
        self._zeros = [
            np.zeros((n_cores * z.shape[0], *z.shape[1:]), z.dtype)
            for z in zero_outs
        ]
        self._dev_args = None

    def prepare(self, in_maps):
        """Stage concatenated inputs, sharded across cores so execution
        never reshards (resharding would ship bytes through the host)."""
        from jax.sharding import NamedSharding, PartitionSpec
        sh = NamedSharding(self._mesh, PartitionSpec("core"))
        concat = [
            np.concatenate([np.asarray(in_maps[c][name])
                            for c in range(self.n_cores)], axis=0)
            for name in self.param_names
        ]
        self._dev_args = [jax.device_put(a, sh) for a in concat + self._zeros]

    def execute(self):
        outs = self._fn(*self._dev_args)
        jax.block_until_ready(outs)
        return outs

    def execute_chain(self, k):
        """Issue k async executions back-to-back, block once at the end.
        Device-side queuing overlaps the per-dispatch host round-trip, so
        wall(k) = floor + k * hw_exec and the slope over k isolates
        hw_exec."""
        outs = None
        for _ in range(k):
            outs = self._fn(*self._dev_args)
        jax.block_until_ready(outs)
        return outs

    def run(self, in_maps):
        self.prepare(in_maps)
        outs = self.execute()
        return [
            {name: np.asarray(outs[i]).reshape(self.n_cores,
                                               *self.out_avals[i].shape)[c]
             for i, name in enumerate(self.out_names)}
            for c in range(self.n_cores)
        ]


def _assemble(results, caps, plans):
    S_seg, KP = caps
    kp_off = np.concatenate([[0], np.cumsum(KP)]).astype(int)
    full = np.empty((T, H), dtype=np.float32)
    for e in range(NCORES):
        r_out = np.asarray(results[e]["out"], dtype=np.float32)
        for s in range(NCORES):
            L = plans[e][s]
            if L:
                full[np.asarray(L, dtype=np.int64)] = \
                    r_out[kp_off[s]:kp_off[s] + len(L)]
    return full


_RUNNERS = {}


def _get_runner(caps):
    if caps not in _RUNNERS:
        nc = _build_moe(caps[0], list(caps[1]))
        _RUNNERS[caps] = _Runner(nc, NCORES)
    return _RUNNERS[caps]


def kernel(hidden_states, top_k_index, top_k_weights, Wg, Wu, Wd):
    in_maps, caps, plans = _route(hidden_states, top_k_index, top_k_weights)
    for e in range(E):
        in_maps[e]["wg"] = np.asarray(Wg[e], dtype=np.float32).astype(ml_dtypes.bfloat16)
        in_maps[e]["wu"] = np.asarray(Wu[e], dtype=np.float32).astype(ml_dtypes.bfloat16)
        in_maps[e]["wd"] = np.asarray(Wd[e], dtype=np.float32).astype(ml_dtypes.bfloat16)
    runner = _get_runner(caps)
    results = runner.run(in_maps)
    return _assemble(results, caps, plans)


# revision 10
# speedup vs baseline: 1.5289x; 1.5289x over previous
"""MoE (top-2 of 8 experts, SwiGLU) kernel for 8 TRN2 NeuronCores.

Expert-parallel, collective-free. Core e holds expert e's weights resident
in SBUF and computes y_t = MLP_e(x_t) * w[e,t] for exactly the tokens
routed to e (host-side gather builds hsTg = hs^T restricted to those
tokens; pad columns are zero with zero combine weight). Each core writes
its [C_pad, H] block of wcg-scaled rows; the host assembly scatter-adds
the two expert contributions per token (16.7 MFLOP, 0.08% of the matmul
work — measured on-device AllToAll combines cost +100-230us of pure
latency because the software collective degrades concurrent compute, far
more than this pointwise add is worth).

All addressing is compile-time and identical across cores (C_pad =
pad128(max tokens per expert)); per-core variation lives in the data.

Gate/up matmuls stream 512 columns wide (PSUM-bank max; measured optimal
~0.53ns/row on HW). The <512-column tail block interleaves the gate/up
accumulation chains to dodge the ~300ns narrow-matmul latency floor.
Weight loads are split into h-quarters and interleaved with the first
block's chains so the PE starts ~12us in. Matmul operands are bf16 (fp32
PSUM accumulation); rel err vs the fp32 reference is ~5e-3.
"""

import numpy as np
import ml_dtypes

import jax
import concourse.bass as bass
import concourse.tile as tile
from concourse import bacc, mybir
from concourse.bass import ts

E, H, I, T, KTOP = 8, 2048, 1408, 4096, 2
NCORES = 8

BF16 = mybir.dt.bfloat16
F32 = mybir.dt.float32


def _ceil128(x):
    return (x + 127) // 128 * 128


def _route(hidden_states, top_k_index, top_k_weights):
    """Host-side routing. Returns per-core in_maps (hsTg, wcg), C_pad, and
    per-core token lists for host assembly."""
    hs = np.asarray(hidden_states, dtype=np.float32)
    idx = np.asarray(top_k_index).astype(np.int64)
    tw = np.asarray(top_k_weights, dtype=np.float32)

    w = np.zeros((E, T), dtype=np.float32)
    tarange = np.arange(T)
    for k in range(KTOP):
        np.add.at(w, (idx[:, k], tarange), tw[:, k])

    toks = [np.where(w[e] > 0)[0] for e in range(E)]
    # a token with both top-k slots on one expert still appears once, with
    # the summed weight; w>0 holds a.s. for uniform(0,1) weights
    C_pad = _ceil128(max(len(t) for t in toks))

    hsT_bf = np.ascontiguousarray(hs.T).astype(ml_dtypes.bfloat16)
    in_maps, plans = [], []
    for e in range(E):
        n = len(toks[e])
        cols = np.zeros(C_pad, dtype=np.int64)
        cols[:n] = toks[e]
        wcg = np.zeros(C_pad, dtype=np.float32)
        wcg[:n] = w[e, toks[e]]
        g = hsT_bf[:, cols]
        g[:, n:] = 0
        in_maps.append({"hsTg": np.ascontiguousarray(g), "wcg": wcg})
        plans.append(toks[e])
    return in_maps, C_pad, plans


def _build_moe(C_pad, h=H, i_sz=I, ncores=NCORES):
    hc, ic2 = h // 128, i_sz // 128
    hh = hc // 4  # h-chunk quarter for interleaved weight loads
    ntiles = C_pad // 128

    blocks = []
    pos = 0
    while C_pad - pos > 512:
        blocks.append((pos, 512))
        pos += 512
    if C_pad - pos:
        blocks.append((pos, C_pad - pos))

    nc = bacc.Bacc("TRN2", target_bir_lowering=False, debug=False,
                   num_devices=ncores)
    hsTg = nc.declare_dram_parameter("hsTg", [h, C_pad], BF16, isOutput=False).ap()
    wg = nc.declare_dram_parameter("wg", [h, i_sz], BF16, isOutput=False).ap()
    wu = nc.declare_dram_parameter("wu", [h, i_sz], BF16, isOutput=False).ap()
    wd = nc.declare_dram_parameter("wd", [i_sz, h], BF16, isOutput=False).ap()
    wcg = nc.declare_dram_parameter("wcg", [C_pad], F32, isOutput=False).ap()
    out = nc.declare_dram_parameter("out", [C_pad, h], BF16, isOutput=True).ap()

    silu = mybir.ActivationFunctionType.Silu

    with tile.TileContext(nc) as tc:
        with (
            tc.tile_pool(name="wpool", bufs=1) as wpool,
            tc.tile_pool(name="hspool", bufs=2) as hspool,
            tc.tile_pool(name="apool", bufs=1) as apool,
            tc.tile_pool(name="stage", bufs=2) as stage,
            tc.tile_pool(name="ypool", bufs=3) as ypool,
            tc.tile_pool(name="pg", bufs=2, space="PSUM") as pg,
            tc.tile_pool(name="pu", bufs=2, space="PSUM") as pu,
            tc.tile_pool(name="py", bufs=4, space="PSUM") as py,
        ):
            # block 0's hidden states first in the DMA queue, then weight
            # halves in the order the first matmul chains consume them.
            (pos0, nb0) = blocks[0]
            hs0 = hspool.tile([128, hc, nb0], BF16, tag="hst")
            nc.sync.dma_start(
                out=hs0[:],
                in_=hsTg[:, pos0:pos0 + nb0].rearrange("(c p) t -> p c t", p=128))

            wg_h = [wpool.tile([128, hh, i_sz], BF16, name=f"wg{i}",
                               tag=f"wg{i}") for i in range(4)]
            wu_h = [wpool.tile([128, hh, i_sz], BF16, name=f"wu{i}",
                               tag=f"wu{i}") for i in range(4)]
            for i in range(4):
                nc.sync.dma_start(
                    out=wg_h[i][:],
                    in_=wg[i * hh * 128:(i + 1) * hh * 128, :]
                    .rearrange("(c p) i -> p c i", p=128))
                nc.sync.dma_start(
                    out=wu_h[i][:],
                    in_=wu[i * hh * 128:(i + 1) * hh * 128, :]
                    .rearrange("(c p) i -> p c i", p=128))
            wd_sb = wpool.tile([128, ic2, h], BF16, tag="wd")
            nc.sync.dma_start(out=wd_sb[:], in_=wd.rearrange("(c p) j -> p c j", p=128))
            wcg_sb = wpool.tile([128, ntiles], F32, tag="wcg")
            nc.sync.dma_start(out=wcg_sb[:], in_=wcg.rearrange("(ct p) -> p ct", p=128))

            for bi, (pos, nb) in enumerate(blocks):
                if bi == 0:
                    hs_t = hs0
                else:
                    hs_t = hspool.tile([128, hc, nb], BF16, tag="hst")
                    nc.sync.dma_start(
                        out=hs_t[:],
                        in_=hsTg[:, pos:pos + nb].rearrange("(c p) t -> p c t", p=128))

                aT = apool.tile([128, ic2, nb], BF16, tag="aT")
                interleave = nb < 512
                for it in range(ic2):
                    psg = pg.tile([128, nb], F32, tag="psg")
                    psu = pu.tile([128, nb], F32, tag="psu")
                    if interleave:
                        for c in range(hc):
                            half, cc = c // hh, c % hh
                            nc.tensor.matmul(
                                psg[:], lhsT=wg_h[half][:, cc, ts(it, 128)],
                                rhs=hs_t[:, c, :],
                                start=(c == 0), stop=(c == hc - 1))
                            nc.tensor.matmul(
                                psu[:], lhsT=wu_h[half][:, cc, ts(it, 128)],
                                rhs=hs_t[:, c, :],
                                start=(c == 0), stop=(c == hc - 1))
                    else:
                        for half in range(4):
                            for cc in range(hh):
                                c = half * hh + cc
                                nc.tensor.matmul(
                                    psg[:], lhsT=wg_h[half][:, cc, ts(it, 128)],
                                    rhs=hs_t[:, c, :],
                                    start=(c == 0), stop=(c == hc - 1))
                            for cc in range(hh):
                                c = half * hh + cc
                                nc.tensor.matmul(
                                    psu[:], lhsT=wu_h[half][:, cc, ts(it, 128)],
                                    rhs=hs_t[:, c, :],
                                    start=(c == 0), stop=(c == hc - 1))
                    sil = stage.tile([128, nb], F32, tag="sil")
                    nc.scalar.activation(out=sil[:], in_=psg[:], func=silu)
                    nc.vector.tensor_mul(aT[:, it, :], sil[:], psu[:])

                for ct in range(nb // 128):
                    gct = pos // 128 + ct
                    g0 = gct * 128
                    y_sb = ypool.tile([128, h], BF16, tag="ysb")
                    for hb in range(h // 512):
                        psy = py.tile([128, 512], F32, tag="psy")
                        for c2 in range(ic2):
                            nc.tensor.matmul(psy[:],
                                             lhsT=aT[:, c2, ts(ct, 128)],
                                             rhs=wd_sb[:, c2, ts(hb, 512)],
                                             start=(c2 == 0),
                                             stop=(c2 == ic2 - 1))
                        nc.vector.tensor_scalar_mul(
                            y_sb[:, ts(hb, 512)], psy[:],
                            wcg_sb[:, gct:gct + 1])
                    nc.sync.dma_start(out=out[g0:g0 + 128, :], in_=y_sb[:])

    nc.compile()
    return nc


class _Runner:
    """Compile once, execute many. Mirrors bass2jax.run_bass_via_pjrt's
    multi-core path but keeps the jitted callable (and device-resident
    inputs) alive so repeat executions skip XLA/NEFF compilation."""

    def __init__(self, nc, n_cores):
        from concourse import bass2jax, mybir as _mybir
        from jax.experimental.shard_map import shard_map
        from jax.sharding import Mesh, PartitionSpec

        bass2jax.install_neuronx_cc_hook()
        partition_name = (nc.partition_id_tensor.name
                          if nc.partition_id_tensor else None)

        in_names, out_names, out_avals, zero_outs = [], [], [], []
        for alloc in nc.m.functions[0].allocations:
            if not isinstance(alloc, _mybir.MemoryLocationSet):
                continue
            name = alloc.memorylocations[0].name
            if alloc.kind == "ExternalInput":
                if name != partition_name:
                    in_names.append(name)
            elif alloc.kind == "ExternalOutput":
                shape = tuple(alloc.tensor_shape)
                dtype = _mybir.dt.np(alloc.dtype)
                out_names.append(name)
                out_avals.append(jax.core.ShapedArray(shape, dtype))
                zero_outs.append(np.zeros(shape, dtype))
        self.n_params = len(in_names)
        self.param_names = list(in_names)
        self.out_names = out_names
        self.out_avals = out_avals
        self.n_cores = n_cores
        all_names = in_names + out_names
        if partition_name is not None:
            all_names.append(partition_name)

        def _body(*args):
            operands = list(args)
            if partition_name is not None:
                operands.append(bass2jax.partition_id_tensor())
            outs = bass2jax._bass_exec_p.bind(
                *operands,
                out_avals=tuple(out_avals),
                in_names=tuple(all_names),
                out_names=tuple(out_names),
                lowering_input_output_aliases=(),
                sim_require_finite=True,
                sim_require_nnan=True,
                nc=nc,
            )
            return tuple(outs)

        devices = jax.devices()[:n_cores]
        assert len(devices) == n_cores
        mesh = Mesh(np.asarray(devices), ("core",))
        n_ops = self.n_params + len(out_names)
        self._body = _body
        self._mesh = mesh
        self._in_specs = (PartitionSpec("core"),) * n_ops
        self._out_specs = (PartitionSpec("core"),) * len(out_names)
        self._fn = jax.jit(
            shard_map(_body, mesh=mesh,
                      in_specs=self._in_specs,
                      out_specs=self._out_specs,
                      check_rep=False),
            keep_unused=True)
        self._zeros = [
            np.zeros((n_cores * z.shape[0], *z.shape[1:]), z.dtype)
            for z in zero_outs
        ]
        self._dev_args = None

    def prepare(self, in_maps):
        """Stage concatenated inputs, sharded across cores so execution
        never reshards (resharding would ship bytes through the host)."""
        from jax.sharding import NamedSharding, PartitionSpec
        sh = NamedSharding(self._mesh, PartitionSpec("core"))
        concat = [
            np.concatenate([np.asarray(in_maps[c][name])
                            for c in range(self.n_cores)], axis=0)
            for name in self.param_names
        ]
        self._dev_args = [jax.device_put(a, sh) for a in concat + self._zeros]

    def execute(self):
        outs = self._fn(*self._dev_args)
        jax.block_until_ready(outs)
        return outs

    def execute_chain(self, k):
        """Issue k async executions back-to-back, block once at the end.
        Device-side queuing overlaps the per-dispatch host round-trip, so
        wall(k) = floor + k * hw_exec and the slope over k isolates
        hw_exec."""
        outs = None
        for _ in range(k):
            outs = self._fn(*self._dev_args)
        jax.block_until_ready(outs)
        return outs

    def run(self, in_maps):
        self.prepare(in_maps)
        outs = self.execute()
        return [
            {name: np.asarray(outs[i]).reshape(self.n_cores,
                                               *self.out_avals[i].shape)[c]
             for i, name in enumerate(self.out_names)}
            for c in range(self.n_cores)
        ]


def _assemble(results, plans):
    full = np.zeros((T, H), dtype=np.float32)
    for e in range(NCORES):
        L = plans[e]
        if len(L):
            r_out = np.asarray(results[e]["out"][:len(L)], dtype=np.float32)
            full[L] += r_out
    return full


_RUNNERS = {}


def _get_runner(C_pad):
    if C_pad not in _RUNNERS:
        nc = _build_moe(C_pad)
        _RUNNERS[C_pad] = _Runner(nc, NCORES)
    return _RUNNERS[C_pad]


def kernel(hidden_states, top_k_index, top_k_weights, Wg, Wu, Wd):
    in_maps, C_pad, plans = _route(hidden_states, top_k_index, top_k_weights)
    for e in range(E):
        in_maps[e]["wg"] = np.asarray(Wg[e], dtype=np.float32).astype(ml_dtypes.bfloat16)
        in_maps[e]["wu"] = np.asarray(Wu[e], dtype=np.float32).astype(ml_dtypes.bfloat16)
        in_maps[e]["wd"] = np.asarray(Wd[e], dtype=np.float32).astype(ml_dtypes.bfloat16)
    runner = _get_runner(C_pad)
    results = runner.run(in_maps)
    return _assemble(results, plans)


# revision 11
# speedup vs baseline: 1.9859x; 1.2989x over previous
"""MoE (top-2 of 8 experts, SwiGLU) kernel for 8 TRN2 NeuronCores.

Expert-parallel, collective-free. Core e holds expert e's weights resident
in SBUF and computes y_t = MLP_e(x_t) * w[e,t] for exactly the tokens
routed to e (host-side gather builds hsTg = hs^T restricted to those
tokens; pad columns are zero with zero combine weight). Each core writes
its [C_pad, H] block of wcg-scaled rows; the host assembly scatter-adds
the two expert contributions per token (16.7 MFLOP, 0.08% of the matmul
work — measured on-device AllToAll combines cost +100-230us of pure
latency because the software collective degrades concurrent compute, far
more than this pointwise add is worth).

All addressing is compile-time and identical across cores (C_pad =
pad128(max tokens per expert)); per-core variation lives in the data.

Gate/up matmuls stream 512 columns wide (PSUM-bank max; measured optimal
~0.53ns/row on HW). The <512-column tail block interleaves the gate/up
accumulation chains to dodge the ~300ns narrow-matmul latency floor.
Weight loads are split into h-quarters and interleaved with the first
block's chains so the PE starts ~12us in. Matmul operands are bf16 (fp32
PSUM accumulation); rel err vs the fp32 reference is ~5e-3.
"""

import numpy as np
import ml_dtypes

import jax
import concourse.bass as bass
import concourse.tile as tile
from concourse import bacc, mybir
from concourse.bass import ts

E, H, I, T, KTOP = 8, 2048, 1408, 4096, 2
NCORES = 8

BF16 = mybir.dt.bfloat16
F32 = mybir.dt.float32


def _ceil128(x):
    return (x + 127) // 128 * 128


def _route(hidden_states, top_k_index, top_k_weights):
    """Host-side routing. Returns per-core in_maps (hsTg, wcg), C_pad, and
    per-core token lists for host assembly."""
    hs = np.asarray(hidden_states, dtype=np.float32)
    idx = np.asarray(top_k_index).astype(np.int64)
    tw = np.asarray(top_k_weights, dtype=np.float32)

    w = np.zeros((E, T), dtype=np.float32)
    tarange = np.arange(T)
    for k in range(KTOP):
        np.add.at(w, (idx[:, k], tarange), tw[:, k])

    toks = [np.where(w[e] > 0)[0] for e in range(E)]
    # a token with both top-k slots on one expert still appears once, with
    # the summed weight; w>0 holds a.s. for uniform(0,1) weights
    C_pad = _ceil128(max(len(t) for t in toks))

    hsT_bf = np.ascontiguousarray(hs.T).astype(ml_dtypes.bfloat16)
    in_maps, plans = [], []
    for e in range(E):
        n = len(toks[e])
        cols = np.zeros(C_pad, dtype=np.int64)
        cols[:n] = toks[e]
        wcg = np.zeros(C_pad, dtype=np.float32)
        wcg[:n] = w[e, toks[e]]
        g = hsT_bf[:, cols]
        g[:, n:] = 0
        in_maps.append({"hsTg": np.ascontiguousarray(g), "wcg": wcg})
        plans.append(toks[e])
    return in_maps, C_pad, plans


def _build_moe(C_pad, h=H, i_sz=I, ncores=NCORES):
    hc, ic2 = h // 128, i_sz // 128
    hh = hc // 4  # h-chunk quarter for interleaved weight loads
    ntiles = C_pad // 128

    blocks = []
    pos = 0
    while C_pad - pos > 512:
        blocks.append((pos, 512))
        pos += 512
    if C_pad - pos:
        blocks.append((pos, C_pad - pos))

    nc = bacc.Bacc("TRN2", target_bir_lowering=False, debug=False,
                   num_devices=ncores)
    hsTg = nc.declare_dram_parameter("hsTg", [h, C_pad], BF16, isOutput=False).ap()
    wg = nc.declare_dram_parameter("wg", [h, i_sz], BF16, isOutput=False).ap()
    wu = nc.declare_dram_parameter("wu", [h, i_sz], BF16, isOutput=False).ap()
    wd = nc.declare_dram_parameter("wd", [i_sz, h], BF16, isOutput=False).ap()
    wcg = nc.declare_dram_parameter("wcg", [C_pad], F32, isOutput=False).ap()
    out = nc.declare_dram_parameter("out", [C_pad, h], BF16, isOutput=True).ap()

    silu = mybir.ActivationFunctionType.Silu

    with tile.TileContext(nc) as tc:
        with (
            tc.tile_pool(name="wpool", bufs=1) as wpool,
            tc.tile_pool(name="hspool", bufs=2) as hspool,
            tc.tile_pool(name="apool", bufs=1) as apool,
            tc.tile_pool(name="stage", bufs=2) as stage,
            tc.tile_pool(name="ypool", bufs=3) as ypool,
            tc.tile_pool(name="pg", bufs=2, space="PSUM") as pg,
            tc.tile_pool(name="pu", bufs=2, space="PSUM") as pu,
            tc.tile_pool(name="py", bufs=4, space="PSUM") as py,
        ):
            # block 0's hidden states first in the DMA queue, then weight
            # halves in the order the first matmul chains consume them.
            (pos0, nb0) = blocks[0]
            hs0 = hspool.tile([128, hc, nb0], BF16, tag="hst")
            nc.sync.dma_start(
                out=hs0[:],
                in_=hsTg[:, pos0:pos0 + nb0].rearrange("(c p) t -> p c t", p=128))

            wg_h = [wpool.tile([128, hh, i_sz], BF16, name=f"wg{i}",
                               tag=f"wg{i}") for i in range(4)]
            wu_h = [wpool.tile([128, hh, i_sz], BF16, name=f"wu{i}",
                               tag=f"wu{i}") for i in range(4)]
            # load order matches the first gate chain (all wg quarters) then
            # the up chain, so the PE never waits past the first ~10us
            for i in range(4):
                nc.sync.dma_start(
                    out=wg_h[i][:],
                    in_=wg[i * hh * 128:(i + 1) * hh * 128, :]
                    .rearrange("(c p) i -> p c i", p=128))
            for i in range(4):
                nc.sync.dma_start(
                    out=wu_h[i][:],
                    in_=wu[i * hh * 128:(i + 1) * hh * 128, :]
                    .rearrange("(c p) i -> p c i", p=128))
            wd_sb = wpool.tile([128, ic2, h], BF16, tag="wd")
            nc.sync.dma_start(out=wd_sb[:], in_=wd.rearrange("(c p) j -> p c j", p=128))
            wcg_sb = wpool.tile([128, ntiles], F32, tag="wcg")
            nc.sync.dma_start(out=wcg_sb[:], in_=wcg.rearrange("(ct p) -> p ct", p=128))

            for bi, (pos, nb) in enumerate(blocks):
                if bi == 0:
                    hs_t = hs0
                else:
                    hs_t = hspool.tile([128, hc, nb], BF16, tag="hst")
                    nc.sync.dma_start(
                        out=hs_t[:],
                        in_=hsTg[:, pos:pos + nb].rearrange("(c p) t -> p c t", p=128))

                aT = apool.tile([128, ic2, nb], BF16, tag="aT")
                interleave = nb < 512
                for it in range(ic2):
                    psg = pg.tile([128, nb], F32, tag="psg")
                    psu = pu.tile([128, nb], F32, tag="psu")
                    if interleave:
                        for c in range(hc):
                            half, cc = c // hh, c % hh
                            nc.tensor.matmul(
                                psg[:], lhsT=wg_h[half][:, cc, ts(it, 128)],
                                rhs=hs_t[:, c, :],
                                start=(c == 0), stop=(c == hc - 1))
                            nc.tensor.matmul(
                                psu[:], lhsT=wu_h[half][:, cc, ts(it, 128)],
                                rhs=hs_t[:, c, :],
                                start=(c == 0), stop=(c == hc - 1))
                    else:
                        for half in range(4):
                            for cc in range(hh):
                                c = half * hh + cc
                                nc.tensor.matmul(
                                    psg[:], lhsT=wg_h[half][:, cc, ts(it, 128)],
                                    rhs=hs_t[:, c, :],
                                    start=(c == 0), stop=(c == hc - 1))
                            for cc in range(hh):
                                c = half * hh + cc
                                nc.tensor.matmul(
                                    psu[:], lhsT=wu_h[half][:, cc, ts(it, 128)],
                                    rhs=hs_t[:, c, :],
                                    start=(c == 0), stop=(c == hc - 1))
                    sil = stage.tile([128, nb], F32, tag="sil")
                    nc.scalar.activation(out=sil[:], in_=psg[:], func=silu)
                    nc.vector.tensor_mul(aT[:, it, :], sil[:], psu[:])

                for ct in range(nb // 128):
                    gct = pos // 128 + ct
                    g0 = gct * 128
                    y_sb = ypool.tile([128, h], BF16, tag="ysb")
                    for hb in range(h // 512):
                        psy = py.tile([128, 512], F32, tag="psy")
                        for c2 in range(ic2):
                            nc.tensor.matmul(psy[:],
                                             lhsT=aT[:, c2, ts(ct, 128)],
                                             rhs=wd_sb[:, c2, ts(hb, 512)],
                                             start=(c2 == 0),
                                             stop=(c2 == ic2 - 1))
                        nc.vector.tensor_scalar_mul(
                            y_sb[:, ts(hb, 512)], psy[:],
                            wcg_sb[:, gct:gct + 1])
                    nc.sync.dma_start(out=out[g0:g0 + 128, :], in_=y_sb[:])

    nc.compile()
    return nc


class _Runner:
    """Compile once, execute many. Mirrors bass2jax.run_bass_via_pjrt's
    multi-core path but keeps the jitted callable (and device-resident
    inputs) alive so repeat executions skip XLA/NEFF compilation."""

    def __init__(self, nc, n_cores):
        from concourse import bass2jax, mybir as _mybir
        from jax.experimental.shard_map import shard_map
        from jax.sharding import Mesh, PartitionSpec

        bass2jax.install_neuronx_cc_hook()
        partition_name = (nc.partition_id_tensor.name
                          if nc.partition_id_tensor else None)

        in_names, out_names, out_avals, zero_outs = [], [], [], []
        for alloc in nc.m.functions[0].allocations:
            if not isinstance(alloc, _mybir.MemoryLocationSet):
                continue
            name = alloc.memorylocations[0].name
            if alloc.kind == "ExternalInput":
                if name != partition_name:
                    in_names.append(name)
            elif alloc.kind == "ExternalOutput":
                shape = tuple(alloc.tensor_shape)
                dtype = _mybir.dt.np(alloc.dtype)
                out_names.append(name)
                out_avals.append(jax.core.ShapedArray(shape, dtype))
                zero_outs.append(np.zeros(shape, dtype))
        self.n_params = len(in_names)
        self.param_names = list(in_names)
        self.out_names = out_names
        self.out_avals = out_avals
        self.n_cores = n_cores
        all_names = in_names + out_names
        if partition_name is not None:
            all_names.append(partition_name)

        def _body(*args):
            operands = list(args)
            if partition_name is not None:
                operands.append(bass2jax.partition_id_tensor())
            outs = bass2jax._bass_exec_p.bind(
                *operands,
                out_avals=tuple(out_avals),
                in_names=tuple(all_names),
                out_names=tuple(out_names),
                lowering_input_output_aliases=(),
                sim_require_finite=True,
                sim_require_nnan=True,
                nc=nc,
            )
            return tuple(outs)

        devices = jax.devices()[:n_cores]
        assert len(devices) == n_cores
        mesh = Mesh(np.asarray(devices), ("core",))
        n_ops = self.n_params + len(out_names)
        self._body = _body
        self._mesh = mesh
        self._in_specs = (PartitionSpec("core"),) * n_ops
        self._out_specs = (PartitionSpec("core"),) * len(out_names)
        self._fn = jax.jit(
            shard_map(_body, mesh=mesh,
                      in_specs=self._in_specs,
                      out_specs=self._out_specs,
                      check_rep=False),
            keep_unused=True)
        self._zeros = [
            np.zeros((n_cores * z.shape[0], *z.shape[1:]), z.dtype)
            for z in zero_outs
        ]
        self._dev_args = None

    def prepare(self, in_maps):
        """Stage concatenated inputs, sharded across cores so execution
        never reshards (resharding would ship bytes through the host)."""
        from jax.sharding import NamedSharding, PartitionSpec
        sh = NamedSharding(self._mesh, PartitionSpec("core"))
        concat = [
            np.concatenate([np.asarray(in_maps[c][name])
                            for c in range(self.n_cores)], axis=0)
            for name in self.param_names
        ]
        self._dev_args = [jax.device_put(a, sh) for a in concat + self._zeros]

    def execute(self):
        outs = self._fn(*self._dev_args)
        jax.block_until_ready(outs)
        return outs

    def execute_chain(self, k):
        """Issue k async executions back-to-back, block once at the end.
        Device-side queuing overlaps the per-dispatch host round-trip, so
        wall(k) = floor + k * hw_exec and the slope over k isolates
        hw_exec."""
        outs = None
        for _ in range(k):
            outs = self._fn(*self._dev_args)
        jax.block_until_ready(outs)
        return outs

    def run(self, in_maps):
        self.prepare(in_maps)
        outs = self.execute()
        return [
            {name: np.asarray(outs[i]).reshape(self.n_cores,
                                               *self.out_avals[i].shape)[c]
             for i, name in enumerate(self.out_names)}
            for c in range(self.n_cores)
        ]


def _assemble(results, plans):
    full = np.zeros((T, H), dtype=np.float32)
    for e in range(NCORES):
        L = plans[e]
        if len(L):
            r_out = np.asarray(results[e]["out"][:len(L)], dtype=np.float32)
            full[L] += r_out
    return full


_RUNNERS = {}


def _get_runner(C_pad):
    if C_pad not in _RUNNERS:
        nc = _build_moe(C_pad)
        _RUNNERS[C_pad] = _Runner(nc, NCORES)
    return _RUNNERS[C_pad]


def kernel(hidden_states, top_k_index, top_k_weights, Wg, Wu, Wd):
    in_maps, C_pad, plans = _route(hidden_states, top_k_index, top_k_weights)
    for e in range(E):
        in_maps[e]["wg"] = np.asarray(Wg[e], dtype=np.float32).astype(ml_dtypes.bfloat16)
        in_maps[e]["wu"] = np.asarray(Wu[e], dtype=np.float32).astype(ml_dtypes.bfloat16)
        in_maps[e]["wd"] = np.asarray(Wd[e], dtype=np.float32).astype(ml_dtypes.bfloat16)
    runner = _get_runner(C_pad)
    results = runner.run(in_maps)
    return _assemble(results, plans)
